# revision 2
# baseline (speedup 1.0000x reference)
"""Causal self-attention (B=4, T=2048, C=1024, H=16, D=64) on 8 trn2 NeuronCores.

Sharding: tensor-parallel over heads. Core g owns heads {2g, 2g+1}:
  - W_attn columns for those heads' q/k/v (128 cols each) -> per-core [1024, 384]
  - W_proj rows for those heads' channels -> per-core [128, 1024]
Each core computes a full [8192, 1024] bf16 partial of the output projection;
the host sums the 8 partials (the "all-reduce" of row-parallel W_proj).

Device layout notes:
  - x is passed as X^T [C, B*T] so q/k projections contract over the partition dim.
  - All matmul operands are bf16 (PSUM accumulation stays fp32).
  - Attention uses the S^T = K @ Q^T formulation: S^T tiles are [k_tok, q_tok]
    so exp(S)*mask and the P^T @ V matmul need no on-chip transposes of P.
  - V is projected token-major (x-chunk stationary, W_v moving) so the PV
    stationary operand [k_tok, d] comes straight out of PSUM - no PE transposes.
  - The softmax normalizer l[q] = sum_k P[k,q] comes from a ones column
    appended to V (stationary operand): one PSUM accumulation yields [y^T ; l].
  - Normalization: DVE reciprocal on the raw l rows, gpsimd partition_broadcast
    to spread 1/l over 64 partitions, then one DVE multiply per head reading
    y^T straight from PSUM.
  - The b-loop is software-pipelined: QKV projection of batch b+1 is
    interleaved with attention q-tiles of batch b so the TensorE stays busy
    while ScalarE works through the exp stream.
"""

import numpy as np

B, T, C, H, D = 4, 2048, 1024, 16, 64
NCORES = 8
BT = B * T                      # 8192
HPC = H // NCORES               # 2 heads per core
CPC = HPC * D                   # 128 channels per core
NC_CHUNKS = C // 128            # 8 contraction chunks of X^T
QW = 512                        # q-tile width (moving dim)
KW = 128                        # k-tile width (S^T partition dim)

_CACHE = {}
LAST_RESULTS = None             # test harness reads exec_time_ns from here


def _build_bass():
    import concourse.bass as bass
    import concourse.mybir as mybir
    import concourse.tile as tile
    from concourse import bacc
    from concourse.masks import make_upper_triangular

    f32 = mybir.dt.float32
    bf16 = mybir.dt.bfloat16
    Exp = mybir.ActivationFunctionType.Exp
    MUL = mybir.AluOpType.mult
    ADD = mybir.AluOpType.add

    nc = bacc.Bacc()
    xt = nc.dram_tensor("xt", [C, BT], bf16, kind="ExternalInput")
    wg = nc.dram_tensor("wg", [C, 3 * CPC], bf16, kind="ExternalInput")
    bg = nc.dram_tensor("bg", [3 * CPC], f32, kind="ExternalInput")
    wp = nc.dram_tensor("wp", [CPC, C], bf16, kind="ExternalInput")
    outp = nc.dram_tensor("outp", [BT, C], bf16, kind="ExternalOutput")

    with tile.TileContext(nc) as tc:
        with (
            tc.tile_pool(name="const", bufs=1) as cpool,
            tc.tile_pool(name="sb", bufs=2) as sb,
            tc.tile_pool(name="ps", bufs=2, space="PSUM") as ps,
        ):
            # ---- constants ----
            # mask[k, q] = 1.0 where q >= k else 0 (upper triangular incl diag)
            maskf = cpool.tile([128, 128], f32, tag="maskf")
            make_upper_triangular(nc, maskf, val=1.0, diag=True)
            mask = cpool.tile([128, 128], bf16, tag="mask")
            nc.vector.tensor_copy(mask, maskf)

            # ---- weights (scalar DMA queue; x tiles go on sync queue) ----
            bias_sb = []
            for grp in range(2):
                bt_ = cpool.tile([128, 1], f32, tag=f"bias{grp}")
                nc.scalar.dma_start(
                    out=bt_,
                    in_=bg[grp * 128:(grp + 1) * 128].rearrange("(p o) -> p o", o=1),
                )
                bias_sb.append(bt_)
            vbias_row = cpool.tile([1, 128], f32, tag="vbrow")
            nc.scalar.dma_start(
                out=vbias_row, in_=bg[256:384].rearrange("(o p) -> o p", o=1)
            )
            # vbias4[p, s, h, d] = b_attn_v[h*64+d] broadcast over partitions
            vbias4 = cpool.tile([128, 4, 2, 64], f32, tag="vbias4")
            for s in range(4):
                nc.gpsimd.partition_broadcast(
                    vbias4[:, s, :, :].rearrange("p h d -> p (h d)"),
                    vbias_row,
                )
            wg_sb = []
            for ci in range(NC_CHUNKS):
                wgt = cpool.tile([128, 3 * CPC], bf16, tag=f"wg{ci}")
                nc.scalar.dma_start(out=wgt, in_=wg[ci * 128:(ci + 1) * 128, :])
                wg_sb.append(wgt)
            wp_sb = cpool.tile([CPC, C], bf16, tag="wp")
            nc.scalar.dma_start(out=wp_sb, in_=wp[:, :])

            qkv = {}       # b -> (qt_sb, kt_sb)
            vaug = {}      # (b, tt) -> va4 [128, 4, 2, 65] tile  ([V_h | 1])
            pending_proj = []   # deferred (b, qt, yt_sb) -> proj runs one qt later

            def qkv_toktile(b, tt):
                """QKV projection for tokens [b*T + tt*QW, +QW)."""
                tok0 = b * T
                if b not in qkv:
                    qt_sb = sb.tile([128, T], bf16, tag="qt", name="qt_sb")
                    kt_sb = sb.tile([128, T], bf16, tag="kt", name="kt_sb")
                    qkv[b] = (qt_sb, kt_sb)
                dests = qkv[b]
                xts = []
                for ci in range(NC_CHUNKS):
                    xtile = sb.tile([128, QW], bf16, tag="xt", bufs=16, name="xtile")
                    nc.sync.dma_start(
                        out=xtile,
                        in_=xt[ci * 128:(ci + 1) * 128,
                               tok0 + tt * QW: tok0 + (tt + 1) * QW],
                    )
                    xts.append(xtile)
                # q^T / k^T: weight-chunk stationary, x moving -> [d, tok]
                for grp in range(2):
                    pqkv = ps.tile([128, QW], f32, tag="mm", name="pqkv")
                    for ci in range(NC_CHUNKS):
                        nc.tensor.matmul(
                            pqkv,
                            wg_sb[ci][:, grp * 128:(grp + 1) * 128],
                            xts[ci],
                            start=(ci == 0),
                            stop=(ci == NC_CHUNKS - 1),
                        )
                    nc.vector.tensor_scalar_add(
                        out=dests[grp][:, tt * QW:(tt + 1) * QW],
                        in0=pqkv,
                        scalar1=bias_sb[grp],
                    )
                # V token-major: x-chunk stationary, W_v moving -> [tok, d]
                vps4 = ps.tile([128, 4, 128], f32, tag="mm", name="vps4")
                for s in range(4):
                    for ci in range(NC_CHUNKS):
                        nc.tensor.matmul(
                            vps4[:, s, :],
                            xts[ci][:, s * 128:(s + 1) * 128],
                            wg_sb[ci][:, 256:384],
                            start=(ci == 0),
                            stop=(ci == NC_CHUNKS - 1),
                        )
                va4 = sb.tile([128, 4, 2, D + 1], bf16, tag="vaug", bufs=9,
                              name="va4")
                nc.vector.scalar_tensor_tensor(
                    out=va4[:, :, :, 0:D],
                    in0=vps4.rearrange("p s (h d) -> p s h d", d=D),
                    scalar=1.0,
                    in1=vbias4,
                    op0=MUL,
                    op1=ADD,
                )
                nc.gpsimd.memset(va4[:, :, :, D:D + 1], 1.0)
                vaug[(b, tt)] = va4

            def attention_qtile(b, qt):
                qt_sb, kt_sb = qkv[b]
                y2 = ps.tile([D + 1, 2, QW], f32, tag="y", bufs=1, name="y2")
                nkt = (qt + 1) * (QW // KW)
                kdiag = qt * (QW // KW)      # first diagonal k-tile
                for kt in range(nkt):
                    diag = kt >= kdiag
                    qoff = (kt - kdiag) * KW if diag else 0
                    w = QW - qoff
                    qsl = slice(qt * QW + qoff, (qt + 1) * QW)
                    ksl = slice(kt * KW, (kt + 1) * KW)
                    st = ps.tile([128, 2, QW], f32, tag="st", name="st")
                    nc.tensor.matmul(
                        st[:, 0, 0:w], kt_sb[0:64, ksl], qt_sb[0:64, qsl]
                    )
                    nc.tensor.matmul(
                        st[:, 1, 0:w], kt_sb[64:128, ksl], qt_sb[64:128, qsl]
                    )
                    p = sb.tile([128, 2, QW], bf16, tag="p", bufs=4, name="p")
                    nc.scalar.activation(
                        p[:, :, 0:w], st[:, :, 0:w], Exp, scale=1.0 / np.sqrt(D)
                    )
                    if diag:
                        nc.vector.tensor_mul(p[:, 0, 0:KW], p[:, 0, 0:KW], mask)
                        nc.vector.tensor_mul(p[:, 1, 0:KW], p[:, 1, 0:KW], mask)
                    va4 = vaug[(b, kt // 4)]
                    s = kt % 4
                    nc.tensor.matmul(
                        y2[:, 0, qoff:QW], va4[:, s, 0, :], p[:, 0, 0:w],
                        start=(kt == 0), stop=(kt == nkt - 1),
                    )
                    nc.tensor.matmul(
                        y2[:, 1, qoff:QW], va4[:, s, 1, :], p[:, 1, 0:w],
                        start=(kt == 0), stop=(kt == nkt - 1),
                    )

                # ---- normalize: y^T * broadcast(1/l) -> yt_sb [128, QW] ----
                rcl = sb.tile([1, 2, QW], f32, tag="rcl", name="rcl")
                nc.vector.reciprocal(rcl, y2[D:D + 1, :, :])
                lbc = sb.tile([64, 2, QW], f32, tag="lbc", name="lbc")
                nc.gpsimd.partition_broadcast(
                    lbc.rearrange("p h q -> p (h q)"),
                    rcl.rearrange("p h q -> p (h q)"),
                )
                yt_sb = sb.tile([128, QW], bf16, tag="yt", name="yt_sb")
                nc.vector.tensor_mul(yt_sb[0:64, :], y2[0:D, 0, :], lbc[:, 0, :])
                nc.vector.tensor_mul(yt_sb[64:128, :], y2[0:D, 1, :], lbc[:, 1, :])

                pending_proj.append((b, qt, yt_sb))

            def proj_qtile(b, qt, yt_sb):
                tok0 = b * T
                for m in range(QW // 128):
                    osb = sb.tile([128, C], bf16, tag="osb", bufs=3, name="osb")
                    for n in range(C // 512):
                        pp = ps.tile([128, 512], f32, tag="mm", name="pp")
                        nc.tensor.matmul(
                            pp, yt_sb[:, m * 128:(m + 1) * 128],
                            wp_sb[:, n * 512:(n + 1) * 512],
                        )
                        if n == 0:
                            nc.scalar.copy(osb[:, n * 512:(n + 1) * 512], pp)
                        else:
                            nc.vector.tensor_copy(osb[:, n * 512:(n + 1) * 512], pp)
                    row0 = tok0 + qt * QW + m * 128
                    nc.gpsimd.dma_start(out=outp[row0:row0 + 128, :], in_=osb)

            # ---- software-pipelined schedule (q-tiles ascending) ----
            NQT = T // QW
            for tt in range(NQT):
                qkv_toktile(0, tt)
            for b in range(B):
                qts = list(range(NQT))
                for i, qt in enumerate(qts):
                    deferred = pending_proj[:]
                    pending_proj.clear()
                    attention_qtile(b, qt)
                    for args in deferred:
                        proj_qtile(*args)
                    if b + 1 < B:
                        qkv_toktile(b + 1, i)
            for args in pending_proj:
                proj_qtile(*args)
            pending_proj.clear()

    nc.finalize()
    return nc


def _get_nc():
    if "nc" not in _CACHE:
        _CACHE["nc"] = _build_bass()
    return _CACHE["nc"]


def kernel(x, W_attn, b_attn, W_proj, b_proj):
    global LAST_RESULTS
    from concourse import bass_utils
    from ml_dtypes import bfloat16

    x = np.asarray(x, dtype=np.float32)
    W_attn = np.asarray(W_attn, dtype=np.float32)
    b_attn = np.asarray(b_attn, dtype=np.float32)
    W_proj = np.asarray(W_proj, dtype=np.float32)
    b_proj = np.asarray(b_proj, dtype=np.float32)

    xt_full = np.ascontiguousarray(x.reshape(BT, C).T).astype(bfloat16)

    in_maps = []
    for g in range(NCORES):
        cols = slice(g * CPC, (g + 1) * CPC)
        wg_g = np.ascontiguousarray(np.concatenate(
            [W_attn[:, cols], W_attn[:, C:][:, cols], W_attn[:, 2 * C:][:, cols]],
            axis=1,
        )).astype(bfloat16)
        bg_g = np.ascontiguousarray(np.concatenate(
            [b_attn[cols], b_attn[C:][cols], b_attn[2 * C:][cols]]
        ))
        wp_g = np.ascontiguousarray(W_proj[cols, :]).astype(bfloat16)
        in_maps.append({"xt": xt_full, "wg": wg_g, "bg": bg_g, "wp": wp_g})

    nc = _get_nc()
    res = bass_utils.run_bass_kernel_spmd(nc, in_maps, core_ids=list(range(NCORES)))
    LAST_RESULTS = res

    acc = np.zeros((BT, C), dtype=np.float64)
    for r_ in res.results:
        acc += r_["outp"].astype(np.float64)
    acc += b_proj
    return acc.astype(np.float32).reshape(B, T, C)


# revision 11
# speedup vs baseline: 1.3407x; 1.3407x over previous
"""Causal self-attention (B=4, T=2048, C=1024, H=16, D=64) on 8 trn2 NeuronCores.

Sharding: tensor-parallel over heads. Core g owns heads {2g, 2g+1}:
  - W_attn columns for those heads' q/k/v (128 cols each) -> per-core [1024, 384]
  - W_proj rows for those heads' channels -> per-core [128, 1024]
Each core computes a full [8192, 1024] bf16 partial of the output projection;
the host sums the 8 partials (the "all-reduce" of row-parallel W_proj).

Device layout notes:
  - x is passed as X^T [C, B*T] so q/k projections contract over the partition dim.
  - All matmul operands are bf16 (PSUM accumulation stays fp32).
  - Attention uses the S^T = K @ Q^T formulation: S^T tiles are [k_tok, q_tok]
    so exp(S)*mask and the P^T @ V matmul need no on-chip transposes of P.
  - V is projected token-major (x-chunk stationary, W_v moving) so the PV
    stationary operand [k_tok, d] comes straight out of PSUM - no PE transposes.
  - The softmax normalizer l[q] = sum_k P[k,q] comes from a ones column
    appended to V (stationary operand): one PSUM accumulation yields [y^T ; l].
  - Normalization: DVE reciprocal on the raw l rows, gpsimd partition_broadcast
    to spread 1/l over 64 partitions, then one DVE multiply per head reading
    y^T straight from PSUM.
  - The b-loop is software-pipelined: QKV projection of batch b+1 is
    interleaved with attention q-tiles of batch b so the TensorE stays busy
    while ScalarE works through the exp stream.
"""

import numpy as np

B, T, C, H, D = 4, 2048, 1024, 16, 64
NCORES = 8
BT = B * T                      # 8192
HPC = H // NCORES               # 2 heads per core
CPC = HPC * D                   # 128 channels per core
NC_CHUNKS = C // 128            # 8 contraction chunks of X^T
QW = 512                        # q-tile width (moving dim)
KW = 128                        # k-tile width (S^T partition dim)

_CACHE = {}
LAST_RESULTS = None             # test harness reads exec_time_ns from here


def _build_bass():
    import concourse.bass as bass
    import concourse.mybir as mybir
    import concourse.tile as tile
    from concourse import bacc
    from concourse.masks import make_upper_triangular

    f32 = mybir.dt.float32
    bf16 = mybir.dt.bfloat16
    Exp = mybir.ActivationFunctionType.Exp
    MUL = mybir.AluOpType.mult
    ADD = mybir.AluOpType.add

    nc = bacc.Bacc()
    xt = nc.dram_tensor("xt", [C, BT], bf16, kind="ExternalInput")
    wg = nc.dram_tensor("wg", [C, 3 * CPC], bf16, kind="ExternalInput")
    bg = nc.dram_tensor("bg", [3 * CPC], f32, kind="ExternalInput")
    wp = nc.dram_tensor("wp", [CPC, C], bf16, kind="ExternalInput")
    outp = nc.dram_tensor("outp", [BT, C], bf16, kind="ExternalOutput")

    with tile.TileContext(nc) as tc:
        with (
            tc.tile_pool(name="const", bufs=1) as cpool,
            tc.tile_pool(name="sb", bufs=2) as sb,
            tc.tile_pool(name="ps", bufs=2, space="PSUM") as ps,
        ):
            # ---- constants ----
            # mask[k, q] = 1.0 where q >= k else 0 (upper triangular incl diag)
            maskf = cpool.tile([128, 128], f32, tag="maskf")
            make_upper_triangular(nc, maskf, val=1.0, diag=True)
            mask = cpool.tile([128, 128], bf16, tag="mask")
            nc.vector.tensor_copy(mask, maskf)

            # ---- weights (scalar DMA queue; x tiles go on sync queue) ----
            wg_sb = []
            for ci in range(NC_CHUNKS):
                wgt = cpool.tile([128, 3 * CPC], bf16, tag=f"wg{ci}")
                nc.scalar.dma_start(out=wgt, in_=wg[ci * 128:(ci + 1) * 128, :])
                wg_sb.append(wgt)
            bias_sb = []
            for grp in range(2):
                bt_ = cpool.tile([128, 1], f32, tag=f"bias{grp}")
                nc.scalar.dma_start(
                    out=bt_,
                    in_=bg[grp * 128:(grp + 1) * 128].rearrange("(p o) -> p o", o=1),
                )
                bias_sb.append(bt_)
            vbias_row = cpool.tile([1, 128], f32, tag="vbrow")
            nc.scalar.dma_start(
                out=vbias_row, in_=bg[256:384].rearrange("(o p) -> o p", o=1)
            )
            # vbias4[p, s, h, d] = b_attn_v[h*64+d] broadcast over partitions
            vbias4 = cpool.tile([128, 4, 2, 64], f32, tag="vbias4")
            for s in range(4):
                nc.gpsimd.partition_broadcast(
                    vbias4[:, s, :, :].rearrange("p h d -> p (h d)"),
                    vbias_row,
                )
            wp_sb = cpool.tile([CPC, C], bf16, tag="wp")
            nc.scalar.dma_start(out=wp_sb, in_=wp[:, :])

            qkv = {}       # b -> (qt_sb, kt_sb)
            vaug = {}      # (b, tt) -> va4 [128, 4, 2, 65] tile  ([V_h | 1])
            pending_proj = []   # deferred (b, qt, yt_sb) -> proj runs one qt later

            def qkv_toktile(b, tt):
                """QKV projection for tokens [b*T + tt*QW, +QW)."""
                tok0 = b * T
                if b not in qkv:
                    qt_sb = sb.tile([128, T], bf16, tag="qt", name="qt_sb")
                    kt_sb = sb.tile([128, T], bf16, tag="kt", name="kt_sb")
                    qkv[b] = (qt_sb, kt_sb)
                dests = qkv[b]
                xts = []
                for ci in range(NC_CHUNKS):
                    xtile = sb.tile([128, QW], bf16, tag="xt", bufs=24, name="xtile")
                    nc.sync.dma_start(
                        out=xtile,
                        in_=xt[ci * 128:(ci + 1) * 128,
                               tok0 + tt * QW: tok0 + (tt + 1) * QW],
                    )
                    xts.append(xtile)
                # q^T / k^T: weight-chunk stationary, x moving -> [d, tok]
                for grp in range(2):
                    pqkv = ps.tile([128, QW], f32, tag="mm", name="pqkv")
                    for ci in range(NC_CHUNKS):
                        nc.tensor.matmul(
                            pqkv,
                            wg_sb[ci][:, grp * 128:(grp + 1) * 128],
                            xts[ci],
                            start=(ci == 0),
                            stop=(ci == NC_CHUNKS - 1),
                        )
                    nc.vector.tensor_scalar_add(
                        out=dests[grp][:, tt * QW:(tt + 1) * QW],
                        in0=pqkv,
                        scalar1=bias_sb[grp],
                    )
                # V token-major: x-chunk stationary, W_v moving -> [tok, d]
                vps4 = ps.tile([128, 4, 128], f32, tag="mm", name="vps4")
                for s in range(4):
                    for ci in range(NC_CHUNKS):
                        nc.tensor.matmul(
                            vps4[:, s, :],
                            xts[ci][:, s * 128:(s + 1) * 128],
                            wg_sb[ci][:, 256:384],
                            start=(ci == 0),
                            stop=(ci == NC_CHUNKS - 1),
                        )
                va4 = sb.tile([128, 4, 2, D + 1], bf16, tag="vaug", bufs=9,
                              name="va4")
                nc.vector.scalar_tensor_tensor(
                    out=va4[:, :, :, 0:D],
                    in0=vps4.rearrange("p s (h d) -> p s h d", d=D),
                    scalar=1.0,
                    in1=vbias4,
                    op0=MUL,
                    op1=ADD,
                )
                nc.gpsimd.memset(va4[:, :, :, D:D + 1], 1.0)
                vaug[(b, tt)] = va4

            def attention_qtile(b, qt):
                qt_sb, kt_sb = qkv[b]
                y2 = ps.tile([128, 2, QW], f32, tag="y", bufs=1, name="y2")
                nkt = (qt + 1) * (QW // KW)
                kdiag = qt * (QW // KW)      # first diagonal k-tile
                for kt in range(nkt):
                    diag = kt >= kdiag
                    qoff = (kt - kdiag) * KW if diag else 0
                    w = QW - qoff
                    qsl = slice(qt * QW + qoff, (qt + 1) * QW)
                    ksl = slice(kt * KW, (kt + 1) * KW)
                    st = ps.tile([128, 2, QW], f32, tag="st", name="st")
                    nc.tensor.matmul(
                        st[:, 0, 0:w], kt_sb[0:64, ksl], qt_sb[0:64, qsl]
                    )
                    nc.tensor.matmul(
                        st[:, 1, 0:w], kt_sb[64:128, ksl], qt_sb[64:128, qsl]
                    )
                    p = sb.tile([128, 2, QW], bf16, tag="p", bufs=4, name="p")
                    nc.scalar.activation(
                        p[:, :, 0:w], st[:, :, 0:w], Exp, scale=1.0 / np.sqrt(D)
                    )
                    if diag:
                        nc.vector.tensor_mul(p[:, 0, 0:KW], p[:, 0, 0:KW], mask)
                        nc.vector.tensor_mul(p[:, 1, 0:KW], p[:, 1, 0:KW], mask)
                    va4 = vaug[(b, kt // 4)]
                    s = kt % 4
                    nc.tensor.matmul(
                        y2[0:D + 1, 0, qoff:QW], va4[:, s, 0, :], p[:, 0, 0:w],
                        start=(kt == 0), stop=(kt == nkt - 1),
                    )
                    nc.tensor.matmul(
                        y2[0:D + 1, 1, qoff:QW], va4[:, s, 1, :], p[:, 1, 0:w],
                        start=(kt == 0), stop=(kt == nkt - 1),
                    )

                # ---- normalize: y^T * broadcast(1/l) -> yt_sb [128, QW] ----
                # Stage y2 to SBUF (frees the PSUM bank), then spread the 1024
                # l-values over 32 partitions with a 32x32 stream-transpose,
                # reciprocal there, transpose back, partition-broadcast.
                ystage = sb.tile([128, 2, QW], f32, tag="ystage", name="ystage")
                nc.vector.tensor_copy(ystage[0:D + 1, :, :], y2[0:D + 1, :, :])
                lrow = ystage.rearrange("p h q -> p (h q)")
                lt = sb.tile([32, 2 * QW], f32, tag="lt", name="lt")
                nc.vector.transpose(lt[0:32, :], lrow[D:D + 32, :])
                rt = sb.tile([32, 2 * QW], f32, tag="rt", name="rt")
                lt_v = lt.rearrange("p (j c) -> p j c", c=32)
                rt_v = rt.rearrange("p (j c) -> p j c", c=32)
                nc.vector.reciprocal(rt_v[:, :, 0:1], lt_v[:, :, 0:1])
                rcf = sb.tile([32, 2 * QW], f32, tag="rcf", name="rcf")
                nc.vector.transpose(rcf[0:32, :], rt[0:32, :])
                lbc = sb.tile([64, 2, QW], f32, tag="lbc", name="lbc")
                nc.gpsimd.partition_broadcast(
                    lbc.rearrange("p h q -> p (h q)"),
                    rcf[0:1, :],
                )
                yt_sb = sb.tile([128, QW], bf16, tag="yt", name="yt_sb")
                nc.vector.tensor_mul(yt_sb[0:64, :], ystage[0:D, 0, :], lbc[:, 0, :])
                nc.vector.tensor_mul(yt_sb[64:128, :], ystage[0:D, 1, :], lbc[:, 1, :])

                pending_proj.append((b, qt, yt_sb))

            def proj_qtile(b, qt, yt_sb):
                tok0 = b * T
                for m in range(QW // 128):
                    osb = sb.tile([128, C], bf16, tag="osb", bufs=3, name="osb")
                    for n in range(C // 512):
                        pp = ps.tile([128, 512], f32, tag="mm", name="pp")
                        nc.tensor.matmul(
                            pp, yt_sb[:, m * 128:(m + 1) * 128],
                            wp_sb[:, n * 512:(n + 1) * 512],
                        )
                        if n == 0:
                            nc.scalar.copy(osb[:, n * 512:(n + 1) * 512], pp)
                        else:
                            nc.vector.tensor_copy(osb[:, n * 512:(n + 1) * 512], pp)
                    row0 = tok0 + qt * QW + m * 128
                    nc.gpsimd.dma_start(out=outp[row0:row0 + 128, :], in_=osb)

            # ---- software-pipelined schedule: qkv units run 2 ahead ----
            NQT = T // QW
            emitted = [0]

            def emit_qkv_until(n):
                while emitted[0] < min(n, B * NQT):
                    ub, ut = divmod(emitted[0], NQT)
                    qkv_toktile(ub, ut)
                    emitted[0] += 1

            emit_qkv_until(2)
            for b in range(B):
                for qt in range(NQT):
                    deferred = pending_proj[:]
                    pending_proj.clear()
                    attention_qtile(b, qt)
                    for args in deferred:
                        proj_qtile(*args)
                    emit_qkv_until(b * NQT + qt + 3)
            for args in pending_proj:
                proj_qtile(*args)
            pending_proj.clear()

    nc.finalize()
    return nc


def _get_nc():
    if "nc" not in _CACHE:
        _CACHE["nc"] = _build_bass()
    return _CACHE["nc"]


def kernel(x, W_attn, b_attn, W_proj, b_proj):
    global LAST_RESULTS
    from concourse import bass_utils
    from ml_dtypes import bfloat16

    x = np.asarray(x, dtype=np.float32)
    W_attn = np.asarray(W_attn, dtype=np.float32)
    b_attn = np.asarray(b_attn, dtype=np.float32)
    W_proj = np.asarray(W_proj, dtype=np.float32)
    b_proj = np.asarray(b_proj, dtype=np.float32)

    xt_full = np.ascontiguousarray(x.reshape(BT, C).T).astype(bfloat16)

    in_maps = []
    for g in range(NCORES):
        cols = slice(g * CPC, (g + 1) * CPC)
        wg_g = np.ascontiguousarray(np.concatenate(
            [W_attn[:, cols], W_attn[:, C:][:, cols], W_attn[:, 2 * C:][:, cols]],
            axis=1,
        )).astype(bfloat16)
        bg_g = np.ascontiguousarray(np.concatenate(
            [b_attn[cols], b_attn[C:][cols], b_attn[2 * C:][cols]]
        ))
        wp_g = np.ascontiguousarray(W_proj[cols, :]).astype(bfloat16)
        in_maps.append({"xt": xt_full, "wg": wg_g, "bg": bg_g, "wp": wp_g})

    nc = _get_nc()
    res = bass_utils.run_bass_kernel_spmd(nc, in_maps, core_ids=list(range(NCORES)))
    LAST_RESULTS = res

    acc = np.zeros((BT, C), dtype=np.float64)
    for r_ in res.results:
        acc += r_["outp"].astype(np.float64)
    acc += b_proj
    return acc.astype(np.float32).reshape(B, T, C)


# revision 14
# speedup vs baseline: 1.3950x; 1.0405x over previous
"""Causal self-attention (B=4, T=2048, C=1024, H=16, D=64) on 8 trn2 NeuronCores.

Sharding: tensor-parallel over heads. Core g owns heads {2g, 2g+1}:
  - W_attn columns for those heads' q/k/v (128 cols each) -> per-core [1024, 384]
  - W_proj rows for those heads' channels -> per-core [128, 1024]
Each core computes a full [8192, 1024] bf16 partial of the output projection;
the host sums the 8 partials (the "all-reduce" of row-parallel W_proj).

Device layout notes:
  - x is passed as X^T [C, B*T] so q/k projections contract over the partition dim.
  - All matmul operands are bf16 (PSUM accumulation stays fp32).
  - Attention uses the S^T = K @ Q^T formulation: S^T tiles are [k_tok, q_tok]
    so exp(S)*mask and the P^T @ V matmul need no on-chip transposes of P.
  - V is projected token-major (x-chunk stationary, W_v moving) so the PV
    stationary operand [k_tok, d] comes straight out of PSUM - no PE transposes.
  - The softmax normalizer l[q] = sum_k P[k,q] comes from a ones column
    appended to V (stationary operand): one PSUM accumulation yields [y^T ; l].
  - Normalization: DVE reciprocal on the raw l rows, gpsimd partition_broadcast
    to spread 1/l over 64 partitions, then one DVE multiply per head reading
    y^T straight from PSUM.
  - The b-loop is software-pipelined: QKV projection of batch b+1 is
    interleaved with attention q-tiles of batch b so the TensorE stays busy
    while ScalarE works through the exp stream.
"""

import numpy as np

B, T, C, H, D = 4, 2048, 1024, 16, 64
NCORES = 8
BT = B * T                      # 8192
HPC = H // NCORES               # 2 heads per core
CPC = HPC * D                   # 128 channels per core
NC_CHUNKS = C // 128            # 8 contraction chunks of X^T
QW = 512                        # q-tile width (moving dim)
KW = 128                        # k-tile width (S^T partition dim)

_CACHE = {}
LAST_RESULTS = None             # test harness reads exec_time_ns from here


def _build_bass():
    import concourse.bass as bass
    import concourse.mybir as mybir
    import concourse.tile as tile
    from concourse import bacc
    from concourse.masks import make_upper_triangular

    f32 = mybir.dt.float32
    bf16 = mybir.dt.bfloat16
    Exp = mybir.ActivationFunctionType.Exp
    MUL = mybir.AluOpType.mult
    ADD = mybir.AluOpType.add

    nc = bacc.Bacc()
    xt = nc.dram_tensor("xt", [C, BT], bf16, kind="ExternalInput")
    wg = nc.dram_tensor("wg", [C, 3 * CPC], bf16, kind="ExternalInput")
    bg = nc.dram_tensor("bg", [3 * CPC], f32, kind="ExternalInput")
    wp = nc.dram_tensor("wp", [CPC, C], bf16, kind="ExternalInput")
    outp = nc.dram_tensor("outp", [BT, C], bf16, kind="ExternalOutput")

    with tile.TileContext(nc) as tc:
        with (
            tc.tile_pool(name="const", bufs=1) as cpool,
            tc.tile_pool(name="sb", bufs=2) as sb,
            tc.tile_pool(name="ps", bufs=2, space="PSUM") as ps,
        ):
            # ---- constants ----
            # mask[k, q] = 1.0 where q >= k else 0 (upper triangular incl diag)
            maskf = cpool.tile([128, 128], f32, tag="maskf")
            make_upper_triangular(nc, maskf, val=1.0, diag=True)
            mask = cpool.tile([128, 128], bf16, tag="mask")
            nc.vector.tensor_copy(mask, maskf)

            # ---- weights (scalar DMA queue; x tiles go on sync queue) ----
            wg_sb = []
            for ci in range(NC_CHUNKS):
                wgt = cpool.tile([128, 3 * CPC], bf16, tag=f"wg{ci}")
                nc.scalar.dma_start(out=wgt, in_=wg[ci * 128:(ci + 1) * 128, :])
                wg_sb.append(wgt)
            bias_sb = []
            for grp in range(2):
                bt_ = cpool.tile([128, 1], f32, tag=f"bias{grp}")
                nc.scalar.dma_start(
                    out=bt_,
                    in_=bg[grp * 128:(grp + 1) * 128].rearrange("(p o) -> p o", o=1),
                )
                bias_sb.append(bt_)
            vbias_row = cpool.tile([1, 128], f32, tag="vbrow")
            nc.scalar.dma_start(
                out=vbias_row, in_=bg[256:384].rearrange("(o p) -> o p", o=1)
            )
            # vbias4[p, s, h, d] = b_attn_v[h*64+d] broadcast over partitions
            vbias4 = cpool.tile([128, 4, 2, 64], f32, tag="vbias4")
            for s in range(4):
                nc.gpsimd.partition_broadcast(
                    vbias4[:, s, :, :].rearrange("p h d -> p (h d)"),
                    vbias_row,
                )
            wp_sb = cpool.tile([CPC, C], bf16, tag="wp")
            nc.scalar.dma_start(out=wp_sb, in_=wp[:, :])

            qkv = {}       # b -> (qt_sb, kt_sb)
            vaug = {}      # (b, tt) -> va4 [128, 4, 2, 65] tile  ([V_h | 1])
            pending_proj = []   # deferred (b, qt, yt_sb) -> proj runs one qt later

            def qkv_chunks(b, tt):
                """QKV projection for tokens [b*T + tt*QW, +QW), as a
                generator yielding after each PE-work chunk (~1us) so the
                chunks can fill exp-wait bubbles inside attention."""
                tok0 = b * T
                if b not in qkv:
                    qt_sb = sb.tile([128, T], bf16, tag="qt", name="qt_sb")
                    kt_sb = sb.tile([128, T], bf16, tag="kt", name="kt_sb")
                    qkv[b] = (qt_sb, kt_sb)
                dests = qkv[b]
                xts = []
                for ci in range(NC_CHUNKS):
                    xtile = sb.tile([128, QW], bf16, tag="xt", bufs=32, name="xtile")
                    nc.sync.dma_start(
                        out=xtile,
                        in_=xt[ci * 128:(ci + 1) * 128,
                               tok0 + tt * QW: tok0 + (tt + 1) * QW],
                    )
                    xts.append(xtile)
                yield
                # q^T / k^T: weight-chunk stationary, x moving -> [d, tok]
                for grp in range(2):
                    pqkv = ps.tile([128, QW], f32, tag="mm", name="pqkv")
                    for ci in range(NC_CHUNKS):
                        nc.tensor.matmul(
                            pqkv,
                            wg_sb[ci][:, grp * 128:(grp + 1) * 128],
                            xts[ci],
                            start=(ci == 0),
                            stop=(ci == NC_CHUNKS - 1),
                        )
                        if ci == 3:
                            yield
                    nc.vector.tensor_scalar_add(
                        out=dests[grp][:, tt * QW:(tt + 1) * QW],
                        in0=pqkv,
                        scalar1=bias_sb[grp],
                    )
                    yield
                # V token-major: x-chunk stationary, W_v moving -> [tok, d]
                vps4 = ps.tile([128, 4, 128], f32, tag="mm", name="vps4")
                for s in range(4):
                    for ci in range(NC_CHUNKS):
                        nc.tensor.matmul(
                            vps4[:, s, :],
                            xts[ci][:, s * 128:(s + 1) * 128],
                            wg_sb[ci][:, 256:384],
                            start=(ci == 0),
                            stop=(ci == NC_CHUNKS - 1),
                        )
                    yield
                va4 = sb.tile([128, 4, 2, D + 1], bf16, tag="vaug", bufs=9,
                              name="va4")
                nc.vector.scalar_tensor_tensor(
                    out=va4[:, :, :, 0:D],
                    in0=vps4.rearrange("p s (h d) -> p s h d", d=D),
                    scalar=1.0,
                    in1=vbias4,
                    op0=MUL,
                    op1=ADD,
                )
                nc.gpsimd.memset(va4[:, :, :, D:D + 1], 1.0)
                vaug[(b, tt)] = va4

            def attention_qtile(b, qt, pull):
                qt_sb, kt_sb = qkv[b]
                y2 = ps.tile([128, 2, QW], f32, tag="y", bufs=1, name="y2")
                nkt = (qt + 1) * (QW // KW)
                kdiag = qt * (QW // KW)      # first diagonal k-tile
                pend = None                  # PV of k-tile kt issues during kt+1

                def issue_pv(kt, p, qoff, w):
                    va4 = vaug[(b, kt // 4)]
                    s = kt % 4
                    nc.tensor.matmul(
                        y2[0:D + 1, 0, qoff:QW], va4[:, s, 0, :], p[:, 0, 0:w],
                        start=(kt == 0), stop=(kt == nkt - 1),
                    )
                    nc.tensor.matmul(
                        y2[0:D + 1, 1, qoff:QW], va4[:, s, 1, :], p[:, 1, 0:w],
                        start=(kt == 0), stop=(kt == nkt - 1),
                    )

                for kt in range(nkt):
                    diag = kt >= kdiag
                    qoff = (kt - kdiag) * KW if diag else 0
                    w = QW - qoff
                    qsl = slice(qt * QW + qoff, (qt + 1) * QW)
                    ksl = slice(kt * KW, (kt + 1) * KW)
                    st = ps.tile([128, 2, QW], f32, tag="st", name="st")
                    nc.tensor.matmul(
                        st[:, 0, 0:w], kt_sb[0:64, ksl], qt_sb[0:64, qsl]
                    )
                    nc.tensor.matmul(
                        st[:, 1, 0:w], kt_sb[64:128, ksl], qt_sb[64:128, qsl]
                    )
                    if pend is not None:
                        issue_pv(*pend)
                    p = sb.tile([128, 2, QW], bf16, tag="p", bufs=4, name="p")
                    nc.scalar.activation(
                        p[:, :, 0:w], st[:, :, 0:w], Exp, scale=1.0 / np.sqrt(D)
                    )
                    if diag:
                        nc.vector.tensor_mul(p[:, 0, 0:KW], p[:, 0, 0:KW], mask)
                        nc.vector.tensor_mul(p[:, 1, 0:KW], p[:, 1, 0:KW], mask)
                    pend = (kt, p, qoff, w)
                    pull(1)
                pull(1)
                issue_pv(*pend)

                # ---- normalize: y^T * broadcast(1/l) -> yt_sb [128, QW] ----
                # Stage y2 to SBUF (frees the PSUM bank), then spread the 1024
                # l-values over 32 partitions with a 32x32 stream-transpose,
                # reciprocal there, transpose back, partition-broadcast.
                ystage = sb.tile([128, 2, QW], f32, tag="ystage", name="ystage")
                nc.vector.tensor_copy(ystage[0:D + 1, :, :], y2[0:D + 1, :, :])
                lrow = ystage.rearrange("p h q -> p (h q)")
                lt = sb.tile([32, 2 * QW], f32, tag="lt", name="lt")
                nc.vector.transpose(lt[0:32, :], lrow[D:D + 32, :])
                rt = sb.tile([32, 2 * QW], f32, tag="rt", name="rt")
                lt_v = lt.rearrange("p (j c) -> p j c", c=32)
                rt_v = rt.rearrange("p (j c) -> p j c", c=32)
                nc.vector.reciprocal(rt_v[:, :, 0:1], lt_v[:, :, 0:1])
                rcf = sb.tile([32, 2 * QW], f32, tag="rcf", name="rcf")
                nc.vector.transpose(rcf[0:32, :], rt[0:32, :])
                lbc = sb.tile([64, 2, QW], f32, tag="lbc", name="lbc")
                nc.gpsimd.partition_broadcast(
                    lbc.rearrange("p h q -> p (h q)"),
                    rcf[0:1, :],
                )
                yt_sb = sb.tile([128, QW], bf16, tag="yt", name="yt_sb")
                nc.vector.tensor_mul(yt_sb[0:64, :], ystage[0:D, 0, :], lbc[:, 0, :])
                nc.vector.tensor_mul(yt_sb[64:128, :], ystage[0:D, 1, :], lbc[:, 1, :])

                pending_proj.append((b, qt, yt_sb))

            def proj_qtile(b, qt, yt_sb):
                tok0 = b * T
                for m in range(QW // 128):
                    osb = sb.tile([128, C], bf16, tag="osb", bufs=3, name="osb")
                    for n in range(C // 512):
                        pp = ps.tile([128, 512], f32, tag="mm", name="pp")
                        nc.tensor.matmul(
                            pp, yt_sb[:, m * 128:(m + 1) * 128],
                            wp_sb[:, n * 512:(n + 1) * 512],
                        )
                        if n == 0:
                            nc.scalar.copy(osb[:, n * 512:(n + 1) * 512], pp)
                        else:
                            nc.vector.tensor_copy(osb[:, n * 512:(n + 1) * 512], pp)
                    row0 = tok0 + qt * QW + m * 128
                    nc.gpsimd.dma_start(out=outp[row0:row0 + 128, :], in_=osb)

            # ---- software-pipelined schedule: qkv units run 2 ahead, and
            # their matmul chunks are pulled as fillers inside attention ----
            NQT = T // QW
            fillers = []        # [unit_idx, generator]
            emitted = [0]

            def add_qkv_gens(n):
                while emitted[0] < min(n, B * NQT):
                    ub, ut = divmod(emitted[0], NQT)
                    fillers.append([emitted[0], qkv_chunks(ub, ut)])
                    emitted[0] += 1

            def pull(n=1):
                done = 0
                while fillers and done < n:
                    try:
                        next(fillers[0][1])
                        done += 1
                    except StopIteration:
                        fillers.pop(0)

            def drain_through(i):
                while fillers and fillers[0][0] <= i:
                    try:
                        next(fillers[0][1])
                    except StopIteration:
                        fillers.pop(0)

            add_qkv_gens(2)
            for b in range(B):
                for qt in range(NQT):
                    i = b * NQT + qt
                    add_qkv_gens(i + 3)
                    drain_through(i)
                    deferred = pending_proj[:]
                    pending_proj.clear()
                    attention_qtile(b, qt, pull)
                    for args in deferred:
                        proj_qtile(*args)
            for args in pending_proj:
                proj_qtile(*args)
            pending_proj.clear()

    nc.finalize()
    return nc


def _get_nc():
    if "nc" not in _CACHE:
        _CACHE["nc"] = _build_bass()
    return _CACHE["nc"]


def kernel(x, W_attn, b_attn, W_proj, b_proj):
    global LAST_RESULTS
    from concourse import bass_utils
    from ml_dtypes import bfloat16

    x = np.asarray(x, dtype=np.float32)
    W_attn = np.asarray(W_attn, dtype=np.float32)
    b_attn = np.asarray(b_attn, dtype=np.float32)
    W_proj = np.asarray(W_proj, dtype=np.float32)
    b_proj = np.asarray(b_proj, dtype=np.float32)

    xt_full = np.ascontiguousarray(x.reshape(BT, C).T).astype(bfloat16)

    in_maps = []
    for g in range(NCORES):
        cols = slice(g * CPC, (g + 1) * CPC)
        wg_g = np.ascontiguousarray(np.concatenate(
            [W_attn[:, cols], W_attn[:, C:][:, cols], W_attn[:, 2 * C:][:, cols]],
            axis=1,
        )).astype(bfloat16)
        bg_g = np.ascontiguousarray(np.concatenate(
            [b_attn[cols], b_attn[C:][cols], b_attn[2 * C:][cols]]
        ))
        wp_g = np.ascontiguousarray(W_proj[cols, :]).astype(bfloat16)
        in_maps.append({"xt": xt_full, "wg": wg_g, "bg": bg_g, "wp": wp_g})

    nc = _get_nc()
    res = bass_utils.run_bass_kernel_spmd(nc, in_maps, core_ids=list(range(NCORES)))
    LAST_RESULTS = res

    acc = np.zeros((BT, C), dtype=np.float64)
    for r_ in res.results:
        acc += r_["outp"].astype(np.float64)
    acc += b_proj
    return acc.astype(np.float32).reshape(B, T, C)
